# revision 9
# baseline (speedup 1.0000x reference)
"""Hawk (RG-LRU) Trainium2 kernel — sequence-sharded, fp8 DoubleRow matmuls.

Sharding (8 cores): core = 2n + s -> batch n in 0..3, time-half s in {0,1}.
Each core computes ALL 1536 channels over its 1024-step half; the recurrence
carry h[T/2-1] crosses the pair via one masked AllReduce of [128,12] f32 and
the second half corrects h += cumprod(alpha) * carry.

All matmuls run as fp8e4m3 DoubleRow (0.5 cyc/row, 2 k-tiles/instr = 4x bf16
MACs/cycle in the cost model) with a hi/lo residual decomposition:
  W @ X ~= Whi@Xhi + Wlo@Xhi + Whi@Xlo    (3 terms = 0.75x bf16 PE cost)
Per k-tile that is 1.5 DoubleRow instructions:
  (Lk|Hk) @ (hi_k|lo_k)   and   (H2j|H2j+1) @ (hi_2j|hi_2j+1)
The forget gate is ~400x attenuated by -8*softplus(forget_base), so its half
of mm2 runs 1-term (cross-pair hi*hi only, 0.25x). Numpy study of this exact
config: rel_err 1.09e-2 (vs 7.3e-3 all-bf16; gate is 2e-2).

Scales (pow2, exact): x*32, w1*512, xh*8, w2*1024, geh*8, w3*1024; unscale
factors fold into existing activation scale params (no extra ops).

Single collective at B-end, hidden under the DEFERRED gate half of mm1
(~31us of PE work vs 28.1us AllReduce latency).
"""

import numpy as np
import ml_dtypes

import concourse.bacc as bacc
import concourse.mybir as mybir
import concourse.tile as tile
from concourse.bass_utils import run_bass_kernel_spmd

f32 = mybir.dt.float32
bf16 = mybir.dt.bfloat16
fp8 = mybir.dt.float8e4
AF = mybir.ActivationFunctionType
ALU = mybir.AluOpType
PM = mybir.MatmulPerfMode
E4 = ml_dtypes.float8_e4m3

S_X, S_W1 = 32.0, 512.0
S_XH, S_W2 = 8.0, 1024.0
S_GEH, S_W3 = 8.0, 1024.0
US1 = 1.0 / (S_X * S_W1)    # mm1 psum unscale (xh copy, gelu)
US2 = 0.5 / (S_XH * S_W2)   # mm2 psum unscale incl the tanh(x/2) halving
US3 = 1.0 / (S_GEH * S_W3)  # mm3 psum unscale


def build_nc(D, HID, T, num_cores=8):
    KD, KH = D // 128, HID // 128      # 8, 12
    MD = D // 128                      # 8 output tiles
    TL = T // 2                        # local timesteps (1024)
    TW = TL + 4                        # x window incl pad/halo (1028)
    NS1 = KD + KD // 2                 # mm1 DR slots per m-tile (12)
    NS2 = KH + KH // 2                 # 3-term DR slots per tile (18)
    NSF = KH // 2                      # forget 1-term slots (6)

    nc = bacc.Bacc("TRN2", target_bir_lowering=False, debug=False,
                   num_devices=num_cores)

    xq_d = nc.dram_tensor("xq", [128, KD, 2, TW], fp8, kind="ExternalInput")
    w1_d = nc.dram_tensor("w1", [2 * KH, 128, NS1, 2, 128], fp8,
                          kind="ExternalInput")
    w2i_d = nc.dram_tensor("w2i", [KH, 128, NS2, 2, 128], fp8,
                           kind="ExternalInput")
    w2f_d = nc.dram_tensor("w2f", [KH, 128, NSF, 2, 128], fp8,
                           kind="ExternalInput")
    w3_d = nc.dram_tensor("w3", [MD, 128, NS2, 2, 128], fp8,
                          kind="ExternalInput")
    cw_d = nc.dram_tensor("cw", [128, KH, 4], f32, kind="ExternalInput")
    cb_d = nc.dram_tensor("cb", [128, KH], f32, kind="ExternalInput")
    gbh_d = nc.dram_tensor("gbh", [128, 2 * KH], f32, kind="ExternalInput")
    pch_d = nc.dram_tensor("pch", [128, KH], f32, kind="ExternalInput")
    msk_d = nc.dram_tensor("msk", [128, 2], f32, kind="ExternalInput")
    out_d = nc.dram_tensor("o", [D, TL], bf16, kind="ExternalOutput")

    RG = [[2 * i, 2 * i + 1] for i in range(num_cores // 2)]

    def mm1_drs(ps, w1s, xq, cs, CA, first_start=True):
        for k in range(KD):
            nc.tensor.matmul(ps[:, 0:CA], w1s[:, k],
                             xq[:, k, :, cs:cs + CA],
                             start=(first_start and k == 0), stop=False,
                             perf_mode=PM.DoubleRow)
        for j in range(KD // 2):
            nc.tensor.matmul(ps[:, 0:CA], w1s[:, KD + j],
                             xq[:, 2 * j:2 * j + 2, 0, cs:cs + CA],
                             start=False, stop=(j == KD // 2 - 1),
                             perf_mode=PM.DoubleRow)

    with tile.TileContext(nc) as tc:
        consts = tc.alloc_tile_pool(name="consts", bufs=1)
        xqp = tc.alloc_tile_pool(name="xqp", bufs=1)
        xip = tc.alloc_tile_pool(name="xi", bufs=KH)
        ppp = tc.alloc_tile_pool(name="pp", bufs=KH)
        carp = tc.alloc_tile_pool(name="car", bufs=2)
        w1p = tc.alloc_tile_pool(name="w1", bufs=3)
        dramp = tc.alloc_tile_pool(name="dram", bufs=2, space="DRAM")
        xhp = tc.alloc_tile_pool(name="xh", bufs=KH)
        xhqp = tc.alloc_tile_pool(name="xhq", bufs=1)

        xq = xqp.tile([128, KD, 2, TW], fp8, tag="xq")
        nc.sync.dma_start(xq[:, :, :, 0:512], xq_d[:, :, :, 0:512])
        w1head = []
        for m in range(3):
            w1m = w1p.tile([128, NS1, 2, 128], fp8, tag="w1", name=f"w1h{m}")
            nc.sync.dma_start(w1m[:], w1_d[m])
            w1head.append(w1m)
        nc.sync.dma_start(xq[:, :, :, 512:1024], xq_d[:, :, :, 512:1024])
        nc.sync.dma_start(xq[:, :, :, 1024:TW], xq_d[:, :, :, 1024:TW])

        cw = consts.tile([128, KH, 4], f32, tag="cw")
        nc.sync.dma_start(cw[:], cw_d[:])
        cb = consts.tile([128, KH], f32, tag="cb")
        nc.sync.dma_start(cb[:], cb_d[:])
        gbh = consts.tile([128, 2 * KH], f32, tag="gbh")
        nc.sync.dma_start(gbh[:], gbh_d[:])
        pch = consts.tile([128, KH], f32, tag="pch")
        nc.sync.dma_start(pch[:], pch_d[:])
        msk = consts.tile([128, 2], f32, tag="msk")
        nc.sync.dma_start(msk[:], msk_d[:])
        qrt = consts.tile([128, 1], f32, tag="qrt")
        nc.gpsimd.memset(qrt[:], 0.25)
        pc2 = consts.tile([128, KH], f32, tag="pc2")
        nc.vector.tensor_scalar(pc2[:], pch[:], 2.0, 0.0, ALU.mult, ALU.add)

        xh = [xhp.tile([128, TW], bf16, tag="xh", name=f"xh{m}")
              for m in range(KH)]
        xhq = xhqp.tile([128, KH, 2, TL], fp8, tag="xhq")
        xi = [xip.tile([128, TL], bf16, tag="xi", name=f"xi{p}")
              for p in range(KH)]
        pp = [ppp.tile([128, TL], bf16, tag="pp", name=f"pp{p}")
              for p in range(KH)]
        carr = carp.tile([128, KH], f32, tag="car", name="carr")
        gat = carp.tile([128, KH], f32, tag="car", name="gat")
        cin_b = dramp.tile([128, KH], f32)
        cout_b = dramp.tile([128, KH], f32)

        # ------------- Phase A: mm1 xh-half + conv + fp8 casts -------------
        # engine constraints (neuron backend): stt/scan are DVE-only; Pool
        # does tt/ts/copy SBUF-only; ACT does activations + PSUM reads.
        N_APFORM = 4   # early tiles run conv as ACT-muls + Pool-adds
        with (
            tc.tile_pool(name="accp", bufs=2) as accp,
            tc.tile_pool(name="prdp", bufs=8) as prdp,
            tc.tile_pool(name="psA", bufs=3, space="PSUM") as psa,
        ):
            for m in range(KH):
                w1m = (w1head[m] if m < 3 else
                       w1p.tile([128, NS1, 2, 128], fp8, tag="w1"))
                if m >= 3:
                    nc.sync.dma_start(w1m[:], w1_d[m])
                pss = []
                for cs, CA in ((0, 512), (512, 512), (1024, 4)):
                    ps = psa.tile([128, 512], f32, tag="psA")
                    mm1_drs(ps, w1m, xq, cs, CA)
                    pss.append(ps)
                nc.scalar.activation(xh[m][:, 0:512], pss[0][:, 0:512],
                                     AF.Copy, scale=US1)
                nc.scalar.activation(xh[m][:, 512:1024], pss[1][:, 0:512],
                                     AF.Copy, scale=US1)
                nc.scalar.activation(xh[m][:, 1024:TW], pss[2][:, 0:4],
                                     AF.Copy, scale=US1)
                if m < N_APFORM:
                    # conv = Σ_s w_s·raw[t-3+s] + b as 4 ACT muls + 3 Pool
                    # adds (keeps DVE free for its exclusive stt/scan work)
                    prods = []
                    for s in range(4):
                        pr = prdp.tile([128, TL], bf16, tag="prd",
                                       name=f"prd{m}_{s}")
                        if s == 0:
                            nc.scalar.activation(pr[:], xh[m][:, 1:1 + TL],
                                                 AF.Identity,
                                                 bias=cb[:, m:m + 1],
                                                 scale=cw[:, m, 0:1])
                        else:
                            nc.scalar.activation(pr[:],
                                                 xh[m][:, 1 + s:1 + s + TL],
                                                 AF.Copy,
                                                 scale=cw[:, m, s:s + 1])
                        prods.append(pr)
                    nc.gpsimd.tensor_add(prods[0][:], prods[0][:], prods[1][:])
                    nc.gpsimd.tensor_add(prods[2][:], prods[2][:], prods[3][:])
                    nc.gpsimd.tensor_add(xh[m][:, 4:4 + TL], prods[0][:],
                                         prods[2][:])
                else:
                    # DVE-form conv in halves (a: out[4:516], b: rest) so the
                    # late tiles' tails resolve quickly for phase B
                    acc = accp.tile([128, TL], bf16, tag="acc")
                    for h0, (rs, os_, L) in enumerate(((1, 4, 512),
                                                       (513, 516, 512))):
                        a = acc[:, h0 * 512:h0 * 512 + L]
                        nc.vector.tensor_scalar(a, xh[m][:, rs:rs + L],
                                                cw[:, m, 0:1], cb[:, m:m + 1],
                                                ALU.mult, ALU.add)
                        for tap in (1, 2):
                            nc.vector.scalar_tensor_tensor(
                                a, xh[m][:, rs + tap:rs + tap + L],
                                cw[:, m, tap:tap + 1], a, ALU.mult, ALU.add)
                    for h0, (rs, os_, L) in enumerate(((1, 4, 512),
                                                       (513, 516, 512))):
                        a = acc[:, h0 * 512:h0 * 512 + L]
                        nc.vector.scalar_tensor_tensor(
                            xh[m][:, os_:os_ + L], xh[m][:, os_:os_ + L],
                            cw[:, m, 3:4], a, ALU.mult, ALU.add)
                # fp8 hi/lo casts into the k-grouped moving tensor
                nc.gpsimd.tensor_scalar(xhq[:, m, 0, :], xh[m][:, 4:4 + TL],
                                        S_XH, 0.0, ALU.mult, ALU.add)
                nc.vector.scalar_tensor_tensor(xhq[:, m, 1, :],
                                               xh[m][:, 4:4 + TL], S_XH,
                                               xhq[:, m, 0, :],
                                               ALU.mult, ALU.subtract)

        # ------------- Phase B: mm2 + gates + local scan -------------
        with (
            tc.tile_pool(name="ttp", bufs=2) as ttp,
            tc.tile_pool(name="alp", bufs=2) as alp,
            tc.tile_pool(name="bsc", bufs=2) as bscp,
            tc.tile_pool(name="bsb", bufs=2) as bsbp,
            tc.tile_pool(name="tip", bufs=2) as tip,
            tc.tile_pool(name="w2ip", bufs=2) as w2ip,
            tc.tile_pool(name="w2fp", bufs=2) as w2fp,
            tc.tile_pool(name="psBf", bufs=2, space="PSUM") as psbf,
            tc.tile_pool(name="psBi", bufs=2, space="PSUM") as psbi,
        ):
            for p in range(KH):
                wf = w2fp.tile([128, NSF, 2, 128], fp8, tag="w2f")
                nc.sync.dma_start(wf[:], w2f_d[p])
                wi = w2ip.tile([128, NS2, 2, 128], fp8, tag="w2i")
                nc.sync.dma_start(wi[:], w2i_d[p])
                psf = psbf.tile([128, TL], f32, tag="psBf")
                for hs in (0, 512):
                    for j in range(NSF):
                        nc.tensor.matmul(
                            psf[:, hs:hs + 512], wf[:, j],
                            xhq[:, 2 * j:2 * j + 2, 0, hs:hs + 512],
                            start=(j == 0), stop=(j == NSF - 1),
                            perf_mode=PM.DoubleRow)
                tt = ttp.tile([128, TL], f32, tag="tt")
                nc.scalar.activation(tt[:], psf[:], AF.Tanh,
                                     bias=gbh[:, p:p + 1], scale=US2)
                alpha = alp.tile([128, TL], f32, tag="alp")
                nc.scalar.activation(alpha[:], tt[:], AF.Exp,
                                     bias=pch[:, p:p + 1],
                                     scale=pch[:, p:p + 1])
                bsc = bscp.tile([128, TL], f32, tag="bsc")
                if p == KH - 1:
                    # carry-critical tile: alpha^2 via a second ACT Exp so the
                    # tail chain stays on one engine
                    nc.scalar.activation(bsc[:], tt[:], AF.Exp,
                                         bias=pc2[:, p:p + 1],
                                         scale=pc2[:, p:p + 1])
                else:
                    nc.gpsimd.tensor_mul(bsc[:], alpha[:], alpha[:])
                bscb = bsbp.tile([128, TL], bf16, tag="bsb")
                nc.scalar.activation(bscb[:], bsc[:], AF.Sqrt,
                                     bias=qrt[:, 0:1], scale=-0.25)
                # pp = use_mask * cumprod(alpha) (identically 0 on even cores)
                nc.vector.tensor_tensor_scan(
                    pp[p][:], alpha[:], alpha[:],
                    msk[:, 1:2], ALU.mult, ALU.bypass)
                nc.vector.tensor_mul(bscb[:], bscb[:], xh[p][:, 4:4 + TL])
                psi = psbi.tile([128, TL], f32, tag="psBi")
                ti = tip.tile([128, TL], bf16, tag="tip")
                for hs in (0, 512):
                    for s in range(KH):
                        nc.tensor.matmul(
                            psi[:, hs:hs + 512], wi[:, s],
                            xhq[:, s, :, hs:hs + 512],
                            start=(s == 0), stop=False,
                            perf_mode=PM.DoubleRow)
                    for j in range(KH // 2):
                        nc.tensor.matmul(
                            psi[:, hs:hs + 512], wi[:, KH + j],
                            xhq[:, 2 * j:2 * j + 2, 0, hs:hs + 512],
                            start=False, stop=(j == KH // 2 - 1),
                            perf_mode=PM.DoubleRow)
                    sl = slice(hs, hs + 512)
                    nc.scalar.activation(ti[:, sl], psi[:, sl], AF.Tanh,
                                         bias=gbh[:, KH + p:KH + p + 1],
                                         scale=US2)
                    nc.vector.scalar_tensor_tensor(
                        xi[p][:, sl], ti[:, sl], 1.0, bscb[:, sl],
                        ALU.add, ALU.mult)
                    nc.vector.tensor_tensor_scan(
                        xi[p][:, sl], alpha[:, sl], xi[p][:, sl],
                        0.0 if hs == 0 else xi[p][:, hs - 1:hs],
                        ALU.mult, ALU.add)
                nc.scalar.activation(carr[:, p:p + 1], xi[p][:, TL - 1:TL],
                                     AF.Copy, scale=msk[:, 0:1])

        # single collective: all 12 carries; in flight under the deferred
        # gate half of mm1 (~31us of PE work)
        nc.gpsimd.dma_start(cin_b[:], carr[:, 0:KH])
        nc.gpsimd.collective_compute(
            "AllReduce", ALU.add, replica_groups=RG,
            ins=[cin_b.opt()], outs=[cout_b.opt()])

        xhqp.release()
        xhp.release()
        gep = tc.alloc_tile_pool(name="ge", bufs=KH)
        ge = [gep.tile([128, TL], bf16, tag="ge", name=f"ge{g}")
              for g in range(KH)]
        gehqp = tc.alloc_tile_pool(name="gehq", bufs=1)
        gehq = gehqp.tile([128, KH, 2, TL], fp8, tag="gehq")

        # ------------- deferred gate half of mm1 + gelu -------------
        with tc.tile_pool(name="psG", bufs=4, space="PSUM") as psg:
            for m in range(KH):
                w1m = w1p.tile([128, NS1, 2, 128], fp8, tag="w1")
                nc.sync.dma_start(w1m[:], w1_d[KH + m])
                for cs in (4, 516):
                    ps = psg.tile([128, 512], f32, tag="psG")
                    mm1_drs(ps, w1m, xq, cs, 512)
                    nc.scalar.activation(ge[m][:, cs - 4:cs - 4 + 512],
                                         ps[:, 0:512], AF.Gelu, scale=US1)

        nc.sync.dma_start(gat[:, 0:KH], cout_b[:])
        # corrections: h = h_local + pp*carry, then geh = ge*h -> fp8 hi/lo
        for q in range(KH):
            nc.vector.scalar_tensor_tensor(xi[q][:], pp[q][:],
                                           gat[:, q:q + 1], xi[q][:],
                                           ALU.mult, ALU.add)
            nc.vector.tensor_mul(ge[q][:], ge[q][:], xi[q][:])
            nc.gpsimd.tensor_scalar(gehq[:, q, 0, :], ge[q][:], S_GEH, 0.0,
                                    ALU.mult, ALU.add)
            nc.vector.scalar_tensor_tensor(gehq[:, q, 1, :], ge[q][:], S_GEH,
                                           gehq[:, q, 0, :],
                                           ALU.mult, ALU.subtract)

        # ------------- Phase C: mm3 -------------
        with (
            tc.tile_pool(name="w3p", bufs=4) as w3p,
            tc.tile_pool(name="outp", bufs=4) as outp,
            tc.tile_pool(name="psC", bufs=2, space="PSUM") as psc,
        ):
            for m in range(MD):
                w3m = w3p.tile([128, NS2, 2, 128], fp8, tag="w3")
                nc.sync.dma_start(w3m[:], w3_d[m])
                ps = psc.tile([128, TL], f32, tag="psC")
                for hs in (0, 512):
                    for s in range(KH):
                        nc.tensor.matmul(
                            ps[:, hs:hs + 512], w3m[:, s],
                            gehq[:, s, :, hs:hs + 512],
                            start=(s == 0), stop=False,
                            perf_mode=PM.DoubleRow)
                    for j in range(KH // 2):
                        nc.tensor.matmul(
                            ps[:, hs:hs + 512], w3m[:, KH + j],
                            gehq[:, 2 * j:2 * j + 2, 0, hs:hs + 512],
                            start=False, stop=(j == KH // 2 - 1),
                            perf_mode=PM.DoubleRow)
                ot = outp.tile([128, TL], bf16, tag="outp")
                for hs in (0, 512):
                    sl = slice(hs, hs + 512)
                    nc.scalar.activation(ot[:, sl], ps[:, sl], AF.Copy,
                                         scale=US3)
                    nc.sync.dma_start(out_d[m * 128:(m + 1) * 128, sl],
                                      ot[:, sl])

        gehqp.release()
        gep.release()
        dramp.release()
        w1p.release()
        carp.release()
        ppp.release()
        xip.release()
        xqp.release()
        consts.release()

    nc.compile()
    return nc


def _hilo(a):
    hi = a.astype(E4)
    lo = (a - hi.astype(np.float32)).astype(E4)
    return hi.astype(np.float32), lo.astype(np.float32)


def _slots3(WT, n_out_tiles, n_k, Hf, Lf):
    """Stationary slot tensors [n_out_tiles, 128, n_k*3//2, 2, 128] from
    quantized hi/lo weight mats (laid out like WT [in, out])."""
    ns = n_k + n_k // 2
    out = np.empty((n_out_tiles, 128, ns, 2, 128), np.float32)
    for m in range(n_out_tiles):
        for k in range(n_k):
            out[m, :, k, 0] = Lf[k * 128:(k + 1) * 128,
                                 m * 128:(m + 1) * 128]
            out[m, :, k, 1] = Hf[k * 128:(k + 1) * 128,
                                 m * 128:(m + 1) * 128]
        for j in range(n_k // 2):
            out[m, :, n_k + j, 0] = Hf[2 * j * 128:(2 * j + 1) * 128,
                                       m * 128:(m + 1) * 128]
            out[m, :, n_k + j, 1] = Hf[(2 * j + 1) * 128:(2 * j + 2) * 128,
                                       m * 128:(m + 1) * 128]
    return out.astype(E4)


def make_in_maps(x, input_w, conv_w, conv_b, gates_w, gates_b, forget_base,
                 output_w, D, HID, T, num_cores):
    KD, KH = D // 128, HID // 128
    MD = D // 128
    N = x.shape[0]
    TL = T // 2

    # mm1: m 0..11 xh half, 12..23 gate half
    w1sel = np.concatenate([input_w[HID:2 * HID], input_w[0:HID]], 0)
    H1, L1 = _hilo(w1sel * S_W1)
    w1q = _slots3(None, 2 * KH, KD, H1.T, L1.T)

    H2, L2 = _hilo(gates_w * S_W2)
    w2i = _slots3(None, KH, KH, H2.T[:, HID:], L2.T[:, HID:])
    # forget half: 1-term, cross-pair hi slots only
    w2f = np.empty((KH, 128, KH // 2, 2, 128), np.float32)
    H2fT = H2.T[:, 0:HID]
    for g in range(KH):
        for j in range(KH // 2):
            w2f[g, :, j, 0] = H2fT[2 * j * 128:(2 * j + 1) * 128,
                                   g * 128:(g + 1) * 128]
            w2f[g, :, j, 1] = H2fT[(2 * j + 1) * 128:(2 * j + 2) * 128,
                                   g * 128:(g + 1) * 128]
    w2f = w2f.astype(E4)

    H3, L3 = _hilo(output_w * S_W3)
    w3q = _slots3(None, MD, KH, H3.T, L3.T)

    cw = np.ascontiguousarray(
        conv_w[:, 0, :].reshape(KH, 128, 4).transpose(1, 0, 2)
    ).astype(np.float32)
    cb = np.ascontiguousarray(conv_b.reshape(KH, 128).T).astype(np.float32)
    gbt = np.ascontiguousarray(
        (0.5 * gates_b).reshape(2 * KH, 128).T).astype(np.float32)
    pcv = (-4.0 * np.log1p(np.exp(forget_base.astype(np.float64))))
    pct = np.ascontiguousarray(pcv.reshape(KH, 128).T).astype(np.float32)

    in_maps = []
    for core in range(num_cores):
        n, s = core // 2, core % 2
        if s == 0:
            win = np.concatenate(
                [np.zeros((4, D), np.float32), x[n, 0:TL]], 0)
        else:
            win = x[n, TL - 4:T]
        ws = win.T * S_X                         # [D, TW]
        hi = ws.astype(E4)
        lo = (ws - hi.astype(np.float32)).astype(E4)
        TW = TL + 4
        xq = np.stack([hi.reshape(KD, 128, TW),
                       lo.reshape(KD, 128, TW)], axis=2)  # [KD,128,2,TW]
        xq = np.ascontiguousarray(xq.transpose(1, 0, 2, 3))
        msk = np.zeros((128, 2), np.float32)
        msk[:, 0] = 1.0 - s   # send mask (even half contributes its carry)
        msk[:, 1] = float(s)  # use mask (odd half applies the carry)
        in_maps.append({
            "xq": xq, "w1": w1q, "w2i": w2i, "w2f": w2f, "w3": w3q,
            "cw": cw, "cb": cb, "gbh": gbt, "pch": pct, "msk": msk,
        })
    return in_maps


_CACHE = {}
TRACE = False
LAST_RES = None


def _get_nc(D, HID, T, num_cores):
    key = (D, HID, T, num_cores)
    if key not in _CACHE:
        _CACHE[key] = build_nc(D, HID, T, num_cores)
    return _CACHE[key]


def run_hawk(x, input_w, conv_w, conv_b, gates_w, gates_b, forget_base,
             output_w, num_cores=8):
    N, T, D = x.shape
    HID = input_w.shape[0] // 2
    nc = _get_nc(D, HID, T, num_cores)
    in_maps = make_in_maps(x, input_w, conv_w, conv_b, gates_w, gates_b,
                           forget_base, output_w, D, HID, T, num_cores)
    global LAST_RES
    res = run_bass_kernel_spmd(nc, in_maps, core_ids=list(range(num_cores)),
                               trace=TRACE)
    LAST_RES = res
    out = np.stack([
        np.concatenate([res.results[2 * n]["o"].astype(np.float32).T,
                        res.results[2 * n + 1]["o"].astype(np.float32).T], 0)
        for n in range(N)
    ])
    return np.ascontiguousarray(out.astype(np.float32))


def kernel(x, input_w, conv_w, conv_b, gates_w, gates_b, forget_base,
           output_w):
    return run_hawk(
        np.asarray(x, dtype=np.float32),
        np.asarray(input_w, dtype=np.float32),
        np.asarray(conv_w, dtype=np.float32),
        np.asarray(conv_b, dtype=np.float32),
        np.asarray(gates_w, dtype=np.float32),
        np.asarray(gates_b, dtype=np.float32),
        np.asarray(forget_base, dtype=np.float32),
        np.asarray(output_w, dtype=np.float32),
    )
